# revision 21
# baseline (speedup 1.0000x reference)
"""BatchedLoRA trn2 kernel: out[t,n,o] = 2.0 * (x @ A[n].T) @ B[n].T.

Sharding: data-parallel over T across 8 cores (1024 tokens each); every core
computes all 8 experts for its token slab.

Pipeline (v7):
  - Full fp16 dataflow (tolerance 2e-2; measured pipeline error ~5e-4):
    x/A/B cast to fp16 on host, mm1/mm2 run fp16 at 1 cyc/col, output
    written fp16 and upcast on host.
  - mm1: adT[128 r, 1024 t] per expert pair, K=2048 accumulated over 16
    k-tiles into [128,512] PSUM groups.
  - mm2: experts 2m/2m+1 run concurrently on disjoint PE row halves
    (K=64 each); 2 matmuls fill a 2-bank [128,1024] PSUM tile.
  - PSUM evictions (the hard bottleneck: every output element must cross
    DVE or ACT at 1 elem/cycle/lane) are assigned least-projected-busy.
  - The PE queue is strictly in-order, so mm2 matmuls stalling on PSUM
    drain would also stall the next pair's mm1. The emitter therefore
    WEAVES mm1(pair p+1) matmuls one-per-PSUM-tile into mm2(pair p),
    across rep boundaries, keeping PE busy during the eviction-limited
    mm2 phase.
  - Output DMAs (1MB, one t-tile x expert pair, 8KB rows) alternate the
    sync HWDGE ring and the gpsimd SWDGE ring, keeping ACT free for
    evictions. Inputs double-buffered so rep r+1 loads overlap rep r.
Host-side prep: transpose x/A/B into SBUF-image layouts, fold the 2.0
scale into B, cast to fp16.
"""
import numpy as np
from contextlib import ExitStack

from concourse import bacc, tile, mybir
from concourse.bass_utils import run_bass_kernel_spmd

# Problem dims (hardcoded per contract)
T, D, DO, R, NE = 8192, 2048, 2048, 64, 8
SCALE = 2.0
N_CORES = 8
TC = T // N_CORES          # tokens per core = 1024
P = 128
KT = D // P                # 16 d-tiles
TCH = TC // 512            # 2 t-chunks of 512 (mm1 moving dim)
NP = NE // 2               # 4 expert pairs
TT = TC // P               # 8 t-chunks of 128 (mm2 stationary dim)
OC = DO // 512             # 4 o-chunks of 512 (mm2 moving dim)
RA = NE * R                # 512 ranks across experts
XC = KT * TC               # 16384 xr columns

F32 = mybir.dt.float32
F16 = mybir.dt.float16
I8 = mybir.dt.int8
# int8 output quantization (variant i8out): out = int8 * (QS/127). QS=2.0
# bounds max|out| (1.713 for the reference distribution).
QS = 2.0


def build_nc(reps: int = 1, variant: str = "full"):
    """Per-core bass program. reps>1 repeats the body for differential timing.

    variants: full (default) | noout | 2ev | 2mm | 2out | i8out | sring |
    ssring | insync | evdve | evact | nowv (no mm1/mm2 weave)."""
    nc = bacc.Bacc("TRN2", target_bir_lowering=False, debug=False)
    i8out = variant == "i8out"
    OT = I8 if i8out else F16
    xh_ap = nc.dram_tensor("xh", [2, P, XC // 2], F16, kind="ExternalInput").ap()
    ah_ap = nc.dram_tensor("ah", [P, KT * RA], F16, kind="ExternalInput").ap()
    bh_ap = nc.dram_tensor("bh", [P, NP * DO], F16, kind="ExternalInput").ap()
    out_ap = nc.dram_tensor("out", [TC, NE, DO], OT, kind="ExternalOutput").ap()

    in_engines = [nc.sync, nc.sync] if variant == "insync" \
        else [nc.sync, nc.scalar]
    out_engines = {
        "ssring": [nc.sync, nc.scalar],
        "sring": [nc.sync],
    }.get(variant, [nc.sync, nc.gpsimd])

    with tile.TileContext(nc) as tc, ExitStack() as ctx:
        xr_p = ctx.enter_context(tc.tile_pool(name="xr", bufs=2))
        ar_p = ctx.enter_context(tc.tile_pool(name="ar", bufs=2))
        br_p = ctx.enter_context(tc.tile_pool(name="br", bufs=2))
        ad_p = ctx.enter_context(tc.tile_pool(name="ad", bufs=3))
        ps1_p = ctx.enter_context(tc.tile_pool(name="ps1", bufs=2, space="PSUM"))
        ps2_p = ctx.enter_context(tc.tile_pool(name="ps2", bufs=3, space="PSUM"))
        os_p = ctx.enter_context(tc.tile_pool(name="os", bufs=4))

        # Eviction (PSUM->SBUF) engine chooser: least-projected-busy of
        # DVE / ACT (gpsimd has no PSUM port). ns-estimate cost model.
        ev_t = {"v": 0.0, "a": 0.0}
        ev_cost = {
            "v": lambda n: (n + 120) / 0.96,
            "a": lambda n: (n + 200) / 1.2,
        }

        def evict(dst, src, n, scale=None):
            if variant == "evdve":
                e = "v"
            elif variant == "evact":
                e = "a"
            else:
                e = min(ev_t, key=lambda k: ev_t[k] + ev_cost[k](n))
            ev_t[e] += ev_cost[e](n)
            if scale is None:
                if e == "v":
                    nc.vector.tensor_copy(dst, src)
                else:
                    nc.scalar.mul(dst, src, 1.0)
            else:
                if e == "v":
                    nc.vector.tensor_scalar_mul(dst, src, scale)
                else:
                    nc.scalar.mul(dst, src, scale)

        tiles = {}   # (rep, m) -> ad tile
        brs = {}     # rep -> br tile

        def emit_inputs(rep):
            xr = xr_p.tile([P, XC], F16, tag="xr", name=f"xr{rep}")
            ar = ar_p.tile([P, KT * RA], F16, tag="ar", name=f"ar{rep}")
            br = br_p.tile([P, NP * DO], F16, tag="br", name=f"br{rep}")
            for g in range(2):
                in_engines[g].dma_start(
                    xr[:, g * (XC // 2):(g + 1) * (XC // 2)], xh_ap[g, :, :])
            in_engines[0].dma_start(ar[:], ah_ap[:, :])
            in_engines[1].dma_start(br[:], bh_ap[:, :])
            brs[rep] = br
            return xr, ar

        def mm1_gen(rep, m, xr, ar):
            """mm1 for pair (rep, m): 32 matmuls, yields after each."""
            ad = ad_p.tile([P, TC], F16, tag="ad", name=f"ad{rep}_{m}")
            tiles[(rep, m)] = ad
            for tch in range(TCH):
                ps = ps1_p.tile([P, 512], F32, tag="ps1",
                                name=f"ps1_{rep}_{m}_{tch}")
                for k in range(KT):
                    nc.tensor.matmul(
                        ps[:],
                        ar[:, k * RA + m * P: k * RA + (m + 1) * P],
                        xr[:, k * TC + tch * 512: k * TC + (tch + 1) * 512],
                        start=(k == 0), stop=(k == KT - 1))
                    if k == KT - 1:
                        evict(ad[:, tch * 512:(tch + 1) * 512], ps[:], 512)
                    yield

        def mm2_gen(rep, m):
            """mm2 for pair (rep, m): yields once per tt group (8 yields).

            Weave bursts land at tt boundaries, where mm2 reloads its
            stationary operand anyway, so woven mm1 matmuls cost no extra
            LDWEIGHTS."""
            ad = tiles.pop((rep, m))
            br = brs[rep]
            for tt in range(TT):
                ot = os_p.tile([P, 2 * DO], OT, tag="os",
                               name=f"os{rep}_{m}_{tt}")
                for half in range(2):
                    for ocp in range(OC // 2):
                        ps = ps2_p.tile([P, 1024], F32, tag="ps2",
                                        name=f"ps2_{rep}_{m}_{tt}_{half}_{ocp}")
                        for oci in range(2):
                            oc = 2 * ocp + oci
                            for dup in range(2 if variant == "2mm" else 1):
                                nc.tensor.matmul(
                                    ps[:, oci * 512:(oci + 1) * 512],
                                    ad[half * 64:(half + 1) * 64,
                                       tt * P:(tt + 1) * P],
                                    br[half * 64:(half + 1) * 64,
                                       m * DO + oc * 512:
                                       m * DO + (oc + 1) * 512],
                                    start=True, stop=True)
                        for dup in range(2 if variant == "2ev" else 1):
                            evict(ot[:, half * DO + ocp * 1024:
                                     half * DO + (ocp + 1) * 1024],
                                  ps[:], 1024,
                                  scale=127.0 / QS if i8out else None)
                yield
                if variant == "noout" and not (m == 0 and tt == 0):
                    continue
                eng = out_engines[(m * TT + tt) % len(out_engines)]
                eng.dma_start(
                    out_ap[tt * P:(tt + 1) * P, 2 * m:2 * m + 2, :], ot[:])
                if variant == "2out":
                    out_engines[(m * TT + tt + 1) % len(out_engines)].dma_start(
                        out_ap[tt * P:(tt + 1) * P, 2 * m:2 * m + 2, :], ot[:])

        # ---- flat pair stream with mm1(p+1) woven into mm2(p) ----
        pairs = [(rep, m) for rep in range(reps) for m in range(NP)]
        xa = emit_inputs(0)
        for _ in mm1_gen(0, 0, *xa):   # head: first pair's mm1 runs unwoven
            pass
        weave = variant != "nowv"
        for idx, (rep, m) in enumerate(pairs):
            nxt = pairs[idx + 1] if idx + 1 < len(pairs) else None
            if nxt is not None:
                if nxt[1] == 0:
                    xa = emit_inputs(nxt[0])
                g1 = mm1_gen(nxt[0], nxt[1], *xa)
            else:
                g1 = None
            for _ in mm2_gen(rep, m):
                if g1 is not None and weave:
                    for _ in range(4):
                        next(g1, None)
            if g1 is not None:
                for _ in g1:
                    pass
    nc.finalize()
    return nc


def make_in_maps(x, A_weights, B_weights):
    xT = np.ascontiguousarray(x.T).astype(np.float16)          # [D, T]
    aT = A_weights.reshape(RA, D).T.astype(np.float16)         # [D, 512]
    b2 = (SCALE * B_weights).astype(np.float16)
    bp = b2.transpose(0, 2, 1).reshape(NP, P, DO)              # expert pairs

    ah = np.ascontiguousarray(
        aT.reshape(KT, P, RA).transpose(1, 0, 2).reshape(P, KT * RA))
    bh = np.ascontiguousarray(
        bp.transpose(1, 0, 2).reshape(P, NP * DO))

    in_maps = []
    for c in range(N_CORES):
        xc = xT[:, c * TC:(c + 1) * TC]                        # [2048, 1024]
        x2 = xc.reshape(KT, P, TC).transpose(1, 0, 2).reshape(P, XC)
        x2 = np.ascontiguousarray(
            x2.reshape(P, 2, XC // 2).transpose(1, 0, 2))      # [2, 128, 8192]
        in_maps.append({"xh": x2, "ah": ah, "bh": bh})
    return in_maps


_NC_CACHE = {}


def kernel(x, A_weights, B_weights):
    x = np.asarray(x, dtype=np.float32)
    A_weights = np.asarray(A_weights, dtype=np.float32)
    B_weights = np.asarray(B_weights, dtype=np.float32)
    if "nc" not in _NC_CACHE:
        _NC_CACHE["nc"] = build_nc(reps=1)
    nc = _NC_CACHE["nc"]
    in_maps = make_in_maps(x, A_weights, B_weights)
    res = run_bass_kernel_spmd(nc, in_maps, list(range(N_CORES)))
    out = np.concatenate([res.results[c]["out"] for c in range(N_CORES)],
                         axis=0)
    return out.astype(np.float32)
